# revision 16
# baseline (speedup 1.0000x reference)
"""Multi-head attention (B=2, S=4096, D=768, H=12, HD=64) on 8 TRN2 NeuronCores.

Sharding: core c handles batch b = c//4 and heads [3*(c%4), 3*(c%4)+3).
Each core computes its 3 heads' attention plus the partial output
projection (row-split Wo); the host sums the 4 partials per batch and
adds bo once.

v2 design (all-bf16 PE path, PE-bound):
  - X is passed as bf16 and transposed by the DMA xbar engine
    (dma_start_transpose), eliminating all PE transposes and staging
    loads.  xT layout: [128, 6, S] with d = p*6 + t; weights are
    host-reshaped to [128, 6, cols] to match.
  - Q/K projections use three 128-wide packed stationaries
    [q01][k01][q2|k2]; head-2 q/k share one [128, S] tile so every
    PSUM->SBUF copy is partition-aligned.
  - V projection is unpadded bf16 [*, 192] with a ones-row bias matmul.
  - scores^T = kT.T @ qT per k-tile ([128, 512] out, f32 PSUM); exp is
    split ScalarE (native Exp) / DVE (Schraudolph int16 bit-trick into
    bf16, zero-mean, cancels in softmax); heads run sequentially so only
    one ctx accumulator bank is held at a time.
  - ctx accumulated with ones-augmented V ([k, 65]); row 64 is the
    softmax denominator; normalize via DVE reciprocal + PE partition
    broadcast.
  - output projection: 3 head matmuls into PSUM; ScalarE copies
    PSUM->SBUF bf16 (no bo on device -- host adds it); DMA out bf16.
"""

import os
import sys

import numpy as np


def _ensure_concourse():
    try:
        import concourse.bass  # noqa: F401
        return
    except ImportError:
        pass
    for p in ("/root/.axon_site/_ro/trn_rl_repo", "/opt/trn_rl_repo"):
        if os.path.isdir(p) and p not in sys.path:
            sys.path.insert(0, p)
    import concourse.bass  # noqa: F401


_ensure_concourse()

import bass_rust  # noqa: E402
import ml_dtypes  # noqa: E402
import concourse.bass as bass  # noqa: E402
import concourse.mybir as mybir  # noqa: E402
import concourse.tile as tile  # noqa: E402

F32 = mybir.dt.float32
F32R = mybir.dt.float32r
BF16 = mybir.dt.bfloat16
I16 = mybir.dt.int16
AF = mybir.ActivationFunctionType
ALU = mybir.AluOpType

B, S, D = 2, 4096, 768
H, HD = 12, 64
NCORES = 8
HPC = 3            # heads per core
SC = 512           # q-chunk width
KG = 2             # k-tiles per exp slab
DT = D // 128      # 6
SCALE = 1.0 / np.sqrt(HD).astype(np.float32)
# Schraudolph exp into bf16 bit pattern: i16 = trunc(score*A16 + B16)
A16 = float(SCALE * np.log2(np.e) * 128.0)
B16 = float((127.0 - 0.0434) * 128.0 + 0.5)
# exp engine per slab g (16 per chunk-head): True -> DVE Schraudolph.
# ScalarE runs Exp ONLY (a Copy interleaved on ScalarE forces an activation
# -table reload, ~1.3us each); DVE takes 6/16 plus all PSUM->SBUF copies.
EXP_DVE = [g in (2, 5, 7, 10, 13, 15) for g in range(16)]


def split_sp_waits(nc, max_waits=1):
    """This walrus build rejects instructions carrying more than one sync
    wait (setupSyncWait "Too many sync wait commands", any engine struct);
    hoist extras onto preceding single-wait NoOps on the same engine."""
    n = 0
    for f in nc.m.functions:
        for blk in f.blocks:
            out = []
            for inst in blk.instructions:
                si = inst.sync_info
                if si is not None and len(si.on_wait) > max_waits:
                    waits = list(si.on_wait)
                    keep, extra = waits[-max_waits:], waits[:-max_waits]
                    for w in extra:
                        n += 1
                        nop = bass_rust.InstNoOp(name=f"I-wsplit-{n}", ins=[], outs=[])
                        nop.engine = inst.engine
                        nop.sync_info = bass_rust.SyncInfo(on_wait=[w], on_update=[])
                        out.append(nop)
                    inst.sync_info = bass_rust.SyncInfo(
                        on_wait=keep, on_update=list(si.on_update))
                out.append(inst)
            blk.instructions = out
    return n


def build_nc(s=S, with_mask=False, split=True, reps=1, parts=None):
    parts = set(parts) if parts is not None else {
        "stageA", "scores", "exp", "ctx", "norm", "outproj"}
    nkt = s // 128          # k tiles
    nch = s // SC           # chunks
    nkg = nkt // KG

    nc = bass.Bass()
    x_d = nc.dram_tensor("x", [s, D], BF16, kind="ExternalInput")
    wqk_d = nc.dram_tensor("wqk", [128, DT, 384], BF16, kind="ExternalInput")
    bqk_d = nc.dram_tensor("bqk", [384], F32, kind="ExternalInput")
    wv_d = nc.dram_tensor("wv", [128, DT, 192], BF16, kind="ExternalInput")
    bv_d = nc.dram_tensor("bv", [192], BF16, kind="ExternalInput")
    wo_d = nc.dram_tensor("wo", [64, HPC, D], BF16, kind="ExternalInput")
    if with_mask:
        mask_d = nc.dram_tensor("mask", [s], F32, kind="ExternalInput")
    out_d = nc.dram_tensor("out", [s, D], BF16, kind="ExternalOutput")

    from concourse.masks import make_identity

    with tile.TileContext(nc) as tc:
        with (
            tc.tile_pool(name="const", bufs=1) as const,
            tc.tile_pool(name="pers", bufs=1) as pers,
            tc.tile_pool(name="xp", bufs=3) as xp,
            tc.tile_pool(name="atp", bufs=4) as atp,
            tc.tile_pool(name="nrm", bufs=2) as nrm,
            tc.tile_pool(name="ctxp", bufs=6) as ctxp,
            tc.tile_pool(name="outp", bufs=3) as outp,
            tc.tile_pool(name="ps", bufs=1, space="PSUM") as ps,
        ):
            # ---- constants / weights (DMA straight in, host pre-layout) ----
            ident = const.tile([128, 128], BF16, name="ident")
            make_identity(nc, ident)
            ones_st = const.tile([1, 64], F32, name="ones_st")
            nc.gpsimd.memset(ones_st, 1.0)
            ones_f = const.tile([1, 64], F32R, name="ones_f")
            nc.vector.tensor_copy(ones_f, ones_st)
            ones_b = const.tile([1, 128], BF16, name="ones_b")
            nc.gpsimd.memset(ones_b, 1.0)

            wqk_sb = const.tile([128, DT, 384], BF16, name="wqk_sb")
            nc.sync.dma_start(out=wqk_sb, in_=wqk_d[:, :, :])
            wv_sb = const.tile([128, DT, 192], BF16, name="wv_sb")
            nc.sync.dma_start(out=wv_sb, in_=wv_d[:, :, :])
            wo_sb = const.tile([64, HPC, D], BF16, name="wo_sb")
            nc.sync.dma_start(out=wo_sb, in_=wo_d[:, :, :])
            bv16 = const.tile([1, 192], BF16, name="bv16")
            nc.sync.dma_start(out=bv16, in_=bv_d[:].rearrange("(o h) -> o h", o=1))

            bq01 = const.tile([128, 1], F32, name="bq01")
            nc.sync.dma_start(out=bq01, in_=bqk_d[0:128].rearrange("(p o) -> p o", o=1))
            bk01 = const.tile([128, 1], F32, name="bk01")
            nc.sync.dma_start(out=bk01, in_=bqk_d[128:256].rearrange("(p o) -> p o", o=1))
            b2 = const.tile([128, 1], F32, name="b2")
            nc.sync.dma_start(out=b2, in_=bqk_d[256:384].rearrange("(p o) -> p o", o=1))

            if with_mask:
                maskb = const.tile([128, nkt], F32, name="maskb")
                nc.sync.dma_start(out=maskb, in_=mask_d[:].rearrange("(t p) -> p t", p=128))
                # bias = (mask - 1) * 1e6   (0 where mask==1, -1e6 where 0)
                nc.vector.tensor_scalar(maskb, maskb, 1.0, 1e6,
                                        ALU.subtract, ALU.mult)

            # ---- persistent activations ----
            xT = pers.tile([128, DT, s], BF16, name="xT")
            qT01 = pers.tile([128, s], BF16, name="qT01")
            kT01 = pers.tile([128, s], BF16, name="kT01")
            qT2 = pers.tile([64, s], BF16, name="qT2")
            kT2 = pers.tile([64, s], BF16, name="kT2")
            v_all = pers.tile([128, nkt, HPC, HD + 1], BF16, name="v_all")
            nc.gpsimd.memset(v_all[:, :, :, HD:HD + 1], 1.0)

            if "ctx" in parts and "exp" not in parts:
                at_const = pers.tile([128, KG * SC], BF16, name="at_const")
                nc.gpsimd.memset(at_const, 1.0)

            for _rep in range(reps):
                # ---- stage A: bf16 PE transposes + projections ----
                for ci in range(nch if "stageA" in parts else 0):
                    cs_, ce_ = ci * SC, (ci + 1) * SC
                    for n in range(SC // 128):
                        st = ci * (SC // 128) + n
                        xarr = xp.tile([128, D], BF16, name="xarr")
                        nc.sync.dma_start(out=xarr, in_=x_d[st * 128:(st + 1) * 128, :])
                        tp = ps.tile([128, D], BF16, tag="acc", bufs=4, name="tp")
                        for t in range(DT):
                            nc.tensor.transpose(tp[:, t * 128:(t + 1) * 128],
                                                xarr[:, t * 128:(t + 1) * 128], ident)
                        nc.vector.tensor_copy(
                            xT[:, :, st * 128:(st + 1) * 128],
                            tp.rearrange("p (t c) -> p t c", t=DT))
                    for g, (dst, b_ap) in enumerate(
                            ((qT01, bq01), (kT01, bk01), (None, b2))):
                        psa = ps.tile([128, SC], F32, tag="acc", bufs=4, name="psa")
                        for t in range(DT):
                            nc.tensor.matmul(psa, wqk_sb[:, t, g * 128:(g + 1) * 128],
                                             xT[:, t, cs_:ce_],
                                             start=(t == 0), stop=(t == DT - 1))
                        if g < 2:
                            nc.vector.tensor_scalar_add(dst[:, cs_:ce_], psa, b_ap)
                        else:
                            # packed [q2|k2] group: k2 copy crosses quadrants
                            # (src parts 64:128 -> dst parts 0:64)
                            nc.vector.tensor_scalar_add(qT2[:, cs_:ce_],
                                                        psa[0:64, :], b_ap[0:64, :])
                            nc.vector.tensor_scalar_add(kT2[:, cs_:ce_],
                                                        psa[64:128, :], b_ap[64:128, :])
                    for n in range(SC // 128):
                        st = ci * (SC // 128) + n
                        vps = ps.tile([128, 192], F32, tag="acc", bufs=4, name="vps")
                        for t in range(DT):
                            nc.tensor.matmul(vps, xT[:, t, st * 128:(st + 1) * 128],
                                             wv_sb[:, t, :],
                                             start=(t == 0), stop=False)
                        nc.tensor.matmul(vps, ones_b[0:1, 0:128], bv16,
                                         start=False, stop=True)
                        nc.vector.tensor_copy(
                            v_all[:, st, :, 0:HD],
                            vps.rearrange("p (h c) -> p h c", h=HPC))

                # ---- stage B/C: attention + output projection, per q-chunk ----
                for j in range(nch if "scores" in parts else 0):
                    cs_list = []
                    for h in range(HPC):
                        if h == 0:
                            kTh, qTh = kT01[0:64, :], qT01[0:64, :]
                        elif h == 1:
                            kTh, qTh = kT01[64:128, :], qT01[64:128, :]
                        else:
                            kTh, qTh = kT2, qT2
                        have_ctx = "ctx" in parts
                        ctx_h = (ps.tile([65, SC], F32, tag="acc", bufs=4, name="ctx_h")
                                 if have_ctx else None)

                        def ctx_emit(g, at):
                            for i in range(KG):
                                kt = g * KG + i
                                nc.tensor.matmul(ctx_h, v_all[:, kt, h, :],
                                                 at[:, i * SC:(i + 1) * SC],
                                                 start=(kt == 0),
                                                 stop=(kt == nkt - 1))

                        # software-pipelined: emit ctx(g-1) after scores(g) so
                        # the in-order PE streams scores while exp engines run
                        pend = None
                        for g in range(nkg):
                            sc_ps = ps.tile([128, KG * SC], F32, tag="big", bufs=2,
                                            name="sc_ps")
                            for i in range(KG):
                                kt = g * KG + i
                                nc.tensor.matmul(sc_ps[:, i * SC:(i + 1) * SC],
                                                 kTh[:, kt * 128:(kt + 1) * 128],
                                                 qTh[:, j * SC:(j + 1) * SC])
                            if "exp" in parts:
                                at = atp.tile([128, KG * SC], BF16, name="at")
                                if with_mask:
                                    for i in range(KG):
                                        kt = g * KG + i
                                        nc.scalar.activation(
                                            at[:, i * SC:(i + 1) * SC],
                                            sc_ps[:, i * SC:(i + 1) * SC],
                                            AF.Exp, bias=maskb[:, kt:kt + 1],
                                            scale=float(SCALE))
                                elif EXP_DVE[(g + 5 * h) % 16]:
                                    nc.vector.tensor_scalar(at.bitcast(I16), sc_ps,
                                                            A16, B16, ALU.mult, ALU.add)
                                else:
                                    nc.scalar.activation(at, sc_ps, AF.Exp,
                                                         scale=float(SCALE))
                            elif have_ctx:
                                at = at_const
                            else:
                                at = None
                            if have_ctx:
                                if pend is not None:
                                    ctx_emit(*pend)
                                pend = (g, at)
                        if have_ctx and pend is not None:
                            ctx_emit(*pend)
                        if not (have_ctx and "norm" in parts):
                            continue
                        # normalize: ctx[0:64] / ctx[64]
                        rs = nrm.tile([1, SC], F32R, name="rs")
                        with nc.allow_low_precision(reason="f32r recip feeds PE broadcast"):
                            nc.vector.reciprocal(rs, ctx_h[64:65, :])
                        rb_ps = ps.tile([64, SC], F32, tag="acc", bufs=4, name="rb_ps")
                        nc.tensor.matmul(rb_ps, ones_f, rs)
                        rb = nrm.tile([64, SC], F32, name="rb")
                        nc.vector.tensor_copy(rb, rb_ps)
                        cs = ctxp.tile([64, SC], BF16, name="cs")
                        nc.vector.tensor_mul(cs, ctx_h[0:64, :], rb)
                        cs_list.append(cs)

                    if not ("norm" in parts and "ctx" in parts
                            and "outproj" in parts):
                        continue
                    # output projection for this chunk's 4 s-tiles
                    for n in range(SC // 128):
                        st = j * (SC // 128) + n
                        osb = outp.tile([128, D], BF16, name="osb")
                        for c0, c1 in ((0, 512), (512, D)):
                            ops = ps.tile([128, c1 - c0], F32, tag="acc", bufs=4,
                                          name="ops")
                            for h in range(HPC):
                                nc.tensor.matmul(ops,
                                                 cs_list[h][:, n * 128:(n + 1) * 128],
                                                 wo_sb[:, h, c0:c1],
                                                 start=(h == 0), stop=(h == HPC - 1))
                            nc.vector.tensor_copy(osb[:, c0:c1], ops)
                        nc.sync.dma_start(out=out_d[st * 128:(st + 1) * 128, :], in_=osb)

    if split:
        split_sp_waits(nc)
    return nc


_BUILD_CACHE = {}


def _get_nc(s, with_mask):
    key = (s, with_mask)
    if key not in _BUILD_CACHE:
        _BUILD_CACHE[key] = build_nc(s, with_mask)
    return _BUILD_CACHE[key]


def make_in_maps(X, mask, Wq, bq, Wk, bk, Wv, bv, Wo, bo, with_mask):
    in_maps = []
    for c in range(NCORES):
        b, hg = divmod(c, 4)
        hsl = slice(HPC * HD * hg, HPC * HD * (hg + 1))
        wq, wk, wv = Wq[:, hsl], Wk[:, hsl], Wv[:, hsl]
        # packed stationary groups: [q01][k01][q2|k2]; the DMA-transposed
        # xT has d = t*128 + p, so weights go [D, c] -> [t, p, c] -> [p, t, c]
        def dtile(w):
            return np.ascontiguousarray(
                w.reshape(DT, 128, w.shape[1]).transpose(1, 0, 2))
        wqk = dtile(np.concatenate(
            [wq[:, 0:128], wk[:, 0:128], wq[:, 128:192], wk[:, 128:192]],
            axis=1))
        bqk = np.concatenate([bq[hsl][0:128], bk[hsl][0:128],
                              bq[hsl][128:192], bk[hsl][128:192]])
        m = {
            "x": np.ascontiguousarray(X[b]).astype(ml_dtypes.bfloat16),
            "wqk": wqk.astype(ml_dtypes.bfloat16),
            "bqk": np.ascontiguousarray(bqk).astype(np.float32),
            "wv": dtile(wv).astype(ml_dtypes.bfloat16),
            "bv": np.ascontiguousarray(bv[hsl]).astype(ml_dtypes.bfloat16),
            "wo": np.ascontiguousarray(
                Wo[hsl, :].reshape(HPC, 64, D).transpose(1, 0, 2)).astype(
                    ml_dtypes.bfloat16),
        }
        if with_mask:
            m["mask"] = np.ascontiguousarray(mask[b]).astype(np.float32)
        in_maps.append(m)
    return in_maps


def kernel(X, mask, Wq, bq, Wk, bk, Wv, bv, Wo, bo):
    from concourse.bass_utils import run_bass_kernel_spmd

    X = np.asarray(X, dtype=np.float32)
    mask = np.asarray(mask, dtype=np.float32)
    Wq, bq = np.asarray(Wq, np.float32), np.asarray(bq, np.float32)
    Wk, bk = np.asarray(Wk, np.float32), np.asarray(bk, np.float32)
    Wv, bv = np.asarray(Wv, np.float32), np.asarray(bv, np.float32)
    Wo, bo = np.asarray(Wo, np.float32), np.asarray(bo, np.float32)

    with_mask = not np.all(mask == 1.0)
    nc = _get_nc(S, with_mask)
    in_maps = make_in_maps(X, mask, Wq, bq, Wk, bk, Wv, bv, Wo, bo, with_mask)
    res = run_bass_kernel_spmd(nc, in_maps, list(range(NCORES))).results
    out = np.zeros((B, S, D), dtype=np.float32)
    for c in range(NCORES):
        out[c // 4] += res[c]["out"].astype(np.float32)
    out += bo
    return out


# revision 31
# speedup vs baseline: 1.2934x; 1.2934x over previous
"""Multi-head attention (B=2, S=4096, D=768, H=12, HD=64) on 8 TRN2 NeuronCores.

Sharding: core c handles batch b = c//4 and heads [3*(c%4), 3*(c%4)+3).
Each core computes its 3 heads' attention plus the partial output
projection (row-split Wo); the host sums the 4 partials per batch and
adds bo once.

v2 design (all-bf16 PE path, PE-bound):
  - X is passed as bf16 and transposed by the DMA xbar engine
    (dma_start_transpose), eliminating all PE transposes and staging
    loads.  xT layout: [128, 6, S] with d = p*6 + t; weights are
    host-reshaped to [128, 6, cols] to match.
  - Q/K projections use three 128-wide packed stationaries
    [q01][k01][q2|k2]; head-2 q/k share one [128, S] tile so every
    PSUM->SBUF copy is partition-aligned.
  - V projection is unpadded bf16 [*, 192] with a ones-row bias matmul.
  - scores^T = kT.T @ qT per k-tile ([128, 512] out, f32 PSUM); exp is
    split ScalarE (native Exp) / DVE (Schraudolph int16 bit-trick into
    bf16, zero-mean, cancels in softmax); heads run sequentially so only
    one ctx accumulator bank is held at a time.
  - ctx accumulated with ones-augmented V ([k, 65]); row 64 is the
    softmax denominator; normalize via DVE reciprocal + PE partition
    broadcast.
  - output projection: 3 head matmuls into PSUM; ScalarE copies
    PSUM->SBUF bf16 (no bo on device -- host adds it); DMA out bf16.
"""

import os
import sys

import numpy as np


def _ensure_concourse():
    try:
        import concourse.bass  # noqa: F401
        return
    except ImportError:
        pass
    for p in ("/root/.axon_site/_ro/trn_rl_repo", "/opt/trn_rl_repo"):
        if os.path.isdir(p) and p not in sys.path:
            sys.path.insert(0, p)
    import concourse.bass  # noqa: F401


_ensure_concourse()

import bass_rust  # noqa: E402
import ml_dtypes  # noqa: E402
import concourse.bass as bass  # noqa: E402
import concourse.mybir as mybir  # noqa: E402
import concourse.tile as tile  # noqa: E402

F32 = mybir.dt.float32
F32R = mybir.dt.float32r
BF16 = mybir.dt.bfloat16
I16 = mybir.dt.int16
AF = mybir.ActivationFunctionType
ALU = mybir.AluOpType

B, S, D = 2, 4096, 768
H, HD = 12, 64
NCORES = 8
HPC = 3            # heads per core
SC = 512           # q-chunk width
KG = 2             # k-tiles per exp slab
DT = D // 128      # 6
SCALE = 1.0 / np.sqrt(HD).astype(np.float32)
# Schraudolph exp into bf16 bit pattern: i16 = trunc(score*A16 + B16)
A16 = float(SCALE * np.log2(np.e) * 128.0)
B16 = float((127.0 - 0.0434) * 128.0 + 0.5)
# exp engine per slab g (16 per chunk-head): True -> DVE Schraudolph.
# ScalarE runs Exp ONLY (a Copy interleaved on ScalarE forces an activation
# -table reload, ~1.3us each); DVE takes 6/16 plus all PSUM->SBUF copies.
EXP_DVE = [g in (2, 5, 7, 10, 13, 15) for g in range(16)]


def split_sp_waits(nc, max_waits=1):
    """This walrus build rejects instructions carrying more than one sync
    wait (setupSyncWait "Too many sync wait commands", any engine struct);
    hoist extras onto preceding single-wait NoOps on the same engine."""
    n = 0
    for f in nc.m.functions:
        for blk in f.blocks:
            out = []
            for inst in blk.instructions:
                si = inst.sync_info
                if si is not None and len(si.on_wait) > max_waits:
                    waits = list(si.on_wait)
                    keep, extra = waits[-max_waits:], waits[:-max_waits]
                    for w in extra:
                        n += 1
                        nop = bass_rust.InstNoOp(name=f"I-wsplit-{n}", ins=[], outs=[])
                        nop.engine = inst.engine
                        nop.sync_info = bass_rust.SyncInfo(on_wait=[w], on_update=[])
                        out.append(nop)
                    inst.sync_info = bass_rust.SyncInfo(
                        on_wait=keep, on_update=list(si.on_update))
                out.append(inst)
            blk.instructions = out
    return n


def build_nc(s=S, with_mask=False, split=True, reps=1, parts=None):
    parts = set(parts) if parts is not None else {
        "stageA", "scores", "exp", "ctx", "norm", "outproj"}
    nkt = s // 128          # k tiles
    nch = s // SC           # chunks
    nkg = nkt // KG

    nc = bass.Bass()
    x_d = nc.dram_tensor("x", [s, D], BF16, kind="ExternalInput")
    wqk_d = nc.dram_tensor("wqk", [128, DT, 384], BF16, kind="ExternalInput")
    bqk_d = nc.dram_tensor("bqk", [384], F32, kind="ExternalInput")
    wv_d = nc.dram_tensor("wv", [128, DT, 192], BF16, kind="ExternalInput")
    bv_d = nc.dram_tensor("bv", [192], BF16, kind="ExternalInput")
    wo_d = nc.dram_tensor("wo", [64, HPC, D], BF16, kind="ExternalInput")
    if with_mask:
        mask_d = nc.dram_tensor("mask", [s], F32, kind="ExternalInput")
    out_d = nc.dram_tensor("out", [s, D], BF16, kind="ExternalOutput")

    from concourse.masks import make_identity

    with tile.TileContext(nc) as tc:
        with (
            tc.tile_pool(name="const", bufs=1) as const,
            tc.tile_pool(name="pers", bufs=1) as pers,
            tc.tile_pool(name="xp", bufs=3) as xp,
            tc.tile_pool(name="atp", bufs=4) as atp,
            tc.tile_pool(name="nrm", bufs=2) as nrm,
            tc.tile_pool(name="ctxp", bufs=6) as ctxp,
            tc.tile_pool(name="outp", bufs=3) as outp,
            tc.tile_pool(name="ps", bufs=1, space="PSUM") as ps,
        ):
            # ---- constants / weights (DMA straight in, host pre-layout) ----
            ident = const.tile([128, 128], BF16, name="ident")
            make_identity(nc, ident)
            ident_f = const.tile([128, 128], F32, name="ident_f")
            make_identity(nc, ident_f)
            ones_st = const.tile([1, 64], F32, name="ones_st")
            nc.gpsimd.memset(ones_st, 1.0)
            ones_b = const.tile([1, 128], BF16, name="ones_b")
            nc.gpsimd.memset(ones_b, 1.0)

            wqk_sb = const.tile([128, DT, 384], BF16, name="wqk_sb")
            nc.sync.dma_start(out=wqk_sb, in_=wqk_d[:, :, :])
            wv_sb = const.tile([128, DT, 192], BF16, name="wv_sb")
            nc.sync.dma_start(out=wv_sb, in_=wv_d[:, :, :])
            wo_sb = const.tile([64, HPC, D], BF16, name="wo_sb")
            nc.sync.dma_start(out=wo_sb, in_=wo_d[:, :, :])
            bv16 = const.tile([1, 192], BF16, name="bv16")
            nc.sync.dma_start(out=bv16, in_=bv_d[:].rearrange("(o h) -> o h", o=1))

            bq01 = const.tile([128, 1], F32, name="bq01")
            nc.sync.dma_start(out=bq01, in_=bqk_d[0:128].rearrange("(p o) -> p o", o=1))
            bk01 = const.tile([128, 1], F32, name="bk01")
            nc.sync.dma_start(out=bk01, in_=bqk_d[128:256].rearrange("(p o) -> p o", o=1))
            b2 = const.tile([128, 1], F32, name="b2")
            nc.sync.dma_start(out=b2, in_=bqk_d[256:384].rearrange("(p o) -> p o", o=1))

            if with_mask:
                maskb = const.tile([128, nkt], F32, name="maskb")
                nc.sync.dma_start(out=maskb, in_=mask_d[:].rearrange("(t p) -> p t", p=128))
                # bias = (mask - 1) * 1e6   (0 where mask==1, -1e6 where 0)
                nc.vector.tensor_scalar(maskb, maskb, 1.0, 1e6,
                                        ALU.subtract, ALU.mult)

            # ---- persistent activations ----
            xT = pers.tile([128, DT, s], BF16, name="xT")
            qT01 = pers.tile([128, s], BF16, name="qT01")
            kT01 = pers.tile([128, s], BF16, name="kT01")
            qT2 = pers.tile([64, s], BF16, name="qT2")
            kT2 = pers.tile([64, s], BF16, name="kT2")
            v_all = pers.tile([128, nkt, HPC, HD + 1], BF16, name="v_all")
            nc.gpsimd.memset(v_all[:, :, :, HD:HD + 1], 1.0)

            if "ctx" in parts and "exp" not in parts:
                at_const = pers.tile([128, KG * SC], BF16, name="at_const")
                nc.gpsimd.memset(at_const, 1.0)

            for _rep in range(reps):
                # ---- stage A: bf16 PE transposes + projections ----
                for ci in range(nch if "stageA" in parts else 0):
                    cs_, ce_ = ci * SC, (ci + 1) * SC
                    for n in range(SC // 128):
                        st = ci * (SC // 128) + n
                        xarr = xp.tile([128, D], BF16, name="xarr")
                        nc.sync.dma_start(out=xarr, in_=x_d[st * 128:(st + 1) * 128, :])
                        tp = ps.tile([128, D], BF16, tag="acc", bufs=4, name="tp")
                        for t in range(DT):
                            nc.tensor.transpose(tp[:, t * 128:(t + 1) * 128],
                                                xarr[:, t * 128:(t + 1) * 128], ident)
                        nc.vector.tensor_copy(
                            xT[:, :, st * 128:(st + 1) * 128],
                            tp.rearrange("p (t c) -> p t c", t=DT))
                    for g, (dst, b_ap) in enumerate(
                            ((qT01, bq01), (kT01, bk01), (None, b2))):
                        psa = ps.tile([128, SC], F32, tag="acc", bufs=4, name="psa")
                        for t in range(DT):
                            nc.tensor.matmul(psa, wqk_sb[:, t, g * 128:(g + 1) * 128],
                                             xT[:, t, cs_:ce_],
                                             start=(t == 0), stop=(t == DT - 1))
                        if g < 2:
                            nc.vector.tensor_scalar_add(dst[:, cs_:ce_], psa, b_ap)
                        else:
                            # packed [q2|k2] group: k2 copy crosses quadrants
                            # (src parts 64:128 -> dst parts 0:64)
                            nc.vector.tensor_scalar_add(qT2[:, cs_:ce_],
                                                        psa[0:64, :], b_ap[0:64, :])
                            nc.vector.tensor_scalar_add(kT2[:, cs_:ce_],
                                                        psa[64:128, :], b_ap[64:128, :])
                    for n in range(SC // 128):
                        st = ci * (SC // 128) + n
                        vps = ps.tile([128, 192], F32, tag="acc", bufs=4, name="vps")
                        for t in range(DT):
                            nc.tensor.matmul(vps, xT[:, t, st * 128:(st + 1) * 128],
                                             wv_sb[:, t, :],
                                             start=(t == 0), stop=False)
                        nc.tensor.matmul(vps, ones_b[0:1, 0:128], bv16,
                                         start=False, stop=True)
                        nc.vector.tensor_copy(
                            v_all[:, st, :, 0:HD],
                            vps.rearrange("p (h c) -> p h c", h=HPC))

                # ---- stage B/C: attention + output projection, per q-chunk ----
                for j in range(nch if "scores" in parts else 0):
                    cs_list = []
                    den = nrm.tile([1, HPC, SC], F32, tag="den", name="den")
                    for h in range(HPC):
                        if h == 0:
                            kTh, qTh = kT01[0:64, :], qT01[0:64, :]
                        elif h == 1:
                            kTh, qTh = kT01[64:128, :], qT01[64:128, :]
                        else:
                            kTh, qTh = kT2, qT2
                        have_ctx = "ctx" in parts
                        ctx_h = (ps.tile([65, SC], F32, tag="acc", bufs=4, name="ctx_h")
                                 if have_ctx else None)

                        def ctx_emit(g, at):
                            for i in range(KG):
                                kt = g * KG + i
                                nc.tensor.matmul(ctx_h, v_all[:, kt, h, :],
                                                 at[:, i * SC:(i + 1) * SC],
                                                 start=(kt == 0),
                                                 stop=(kt == nkt - 1))

                        # software-pipelined: emit ctx(g-1) after scores(g) so
                        # the in-order PE streams scores while exp engines run
                        pend = None
                        for g in range(nkg):
                            sc_ps = ps.tile([128, KG * SC], F32, tag="big", bufs=2,
                                            name="sc_ps")
                            for i in range(KG):
                                kt = g * KG + i
                                nc.tensor.matmul(sc_ps[:, i * SC:(i + 1) * SC],
                                                 kTh[:, kt * 128:(kt + 1) * 128],
                                                 qTh[:, j * SC:(j + 1) * SC])
                            if "exp" in parts:
                                at = atp.tile([128, KG * SC], BF16, name="at")
                                if with_mask:
                                    for i in range(KG):
                                        kt = g * KG + i
                                        nc.scalar.activation(
                                            at[:, i * SC:(i + 1) * SC],
                                            sc_ps[:, i * SC:(i + 1) * SC],
                                            AF.Exp, bias=maskb[:, kt:kt + 1],
                                            scale=float(SCALE))
                                elif EXP_DVE[(g + 5 * h) % 16]:
                                    nc.vector.tensor_scalar(at.bitcast(I16), sc_ps,
                                                            A16, B16, ALU.mult, ALU.add)
                                else:
                                    nc.scalar.activation(at, sc_ps, AF.Exp,
                                                         scale=float(SCALE))
                            elif have_ctx:
                                at = at_const
                            else:
                                at = None
                            if have_ctx:
                                if pend is not None:
                                    ctx_emit(*pend)
                                pend = (g, at)
                        if have_ctx and pend is not None:
                            ctx_emit(*pend)
                        if not have_ctx or not parts & {"norm", "dummynorm", "norecip"}:
                            continue
                        if "dummynorm" in parts:
                            cs = ctxp.tile([64, SC], BF16, name="cs")
                            nc.gpsimd.memset(cs, 1.0)
                            cs_list.append(cs)
                            continue
                        # stash unnormalized ctx + denominator row; the
                        # reciprocals for all 3 heads run back-to-back on
                        # ScalarE at chunk end (2 act-table swaps per chunk,
                        # not 2 per head; DVE's Reciprocal is ~10us a call)
                        nc.vector.tensor_copy(den[:, h, :], ctx_h[64:65, :])
                        csu = ctxp.tile([64, SC], BF16, tag="csu", name="csu")
                        nc.vector.tensor_copy(csu, ctx_h[0:64, :])
                        cs_list.append((csu, den))

                    if not (parts & {"norm", "dummynorm", "norecip"}
                            and "ctx" in parts and "outproj" in parts):
                        continue
                    if "dummynorm" not in parts:
                        # reciprocal is ~20 cyc per element PER LANE on DVE;
                        # transpose the 3x512 denominators to [128, 12] (12
                        # elements/lane), recip there, transpose back
                        nblk = HPC * SC // 128  # 12
                        denv = den.rearrange("o h (x c) -> o (h x) c", c=128)
                        dpt = ps.tile([128, nblk], F32, tag="acc", bufs=4,
                                      name="dpt")
                        for m in range(nblk):
                            nc.tensor.transpose(dpt[:, m:m + 1], denv[:, m, :],
                                                ident_f[0:1, 0:1])
                        rcp = nrm.tile([128, nblk], F32, tag="rcp", name="rcp")
                        if "norecip" in parts:
                            nc.gpsimd.memset(rcp, 1.0)
                        else:
                            nc.vector.reciprocal(rcp, dpt)
                        rs_sb = nrm.tile([1, HPC, SC], F32, tag="rs", name="rs_sb")
                        for h in range(HPC):
                            rps = ps.tile([1, SC], F32, tag="acc", bufs=4,
                                          name="rps")
                            rpsv = rps.rearrange("o (m c) -> o m c", c=128)
                            for x in range(SC // 128):
                                m = h * (SC // 128) + x
                                nc.tensor.transpose(rpsv[:, x, :], rcp[:, m:m + 1],
                                                    ident_f)
                            nc.vector.tensor_copy(rs_sb[:, h, :], rps)
                        normed = []
                        for h, (csu, _den) in enumerate(cs_list):
                            rb_ps = ps.tile([64, SC], F32, tag="acc", bufs=4,
                                            name="rb_ps")
                            nc.tensor.matmul(rb_ps, ones_st, rs_sb[:, h, :])
                            rb = nrm.tile([64, SC], F32, tag="rb", name="rb")
                            nc.vector.tensor_copy(rb, rb_ps)
                            cs = ctxp.tile([64, SC], BF16, name="cs")
                            nc.vector.tensor_mul(cs, csu, rb)
                            normed.append(cs)
                        cs_list = normed
                    # output projection for this chunk's 4 s-tiles
                    for n in range(SC // 128):
                        st = j * (SC // 128) + n
                        osb = outp.tile([128, D], BF16, name="osb")
                        for c0, c1 in ((0, 512), (512, D)):
                            ops = ps.tile([128, c1 - c0], F32, tag="acc", bufs=4,
                                          name="ops")
                            for h in range(HPC):
                                nc.tensor.matmul(ops,
                                                 cs_list[h][:, n * 128:(n + 1) * 128],
                                                 wo_sb[:, h, c0:c1],
                                                 start=(h == 0), stop=(h == HPC - 1))
                            nc.vector.tensor_copy(osb[:, c0:c1], ops)
                        nc.sync.dma_start(out=out_d[st * 128:(st + 1) * 128, :], in_=osb)

    if split:
        split_sp_waits(nc)
    return nc


_BUILD_CACHE = {}


def _get_nc(s, with_mask):
    key = (s, with_mask)
    if key not in _BUILD_CACHE:
        _BUILD_CACHE[key] = build_nc(s, with_mask)
    return _BUILD_CACHE[key]


def make_in_maps(X, mask, Wq, bq, Wk, bk, Wv, bv, Wo, bo, with_mask):
    in_maps = []
    for c in range(NCORES):
        b, hg = divmod(c, 4)
        hsl = slice(HPC * HD * hg, HPC * HD * (hg + 1))
        wq, wk, wv = Wq[:, hsl], Wk[:, hsl], Wv[:, hsl]
        # packed stationary groups: [q01][k01][q2|k2]; the DMA-transposed
        # xT has d = t*128 + p, so weights go [D, c] -> [t, p, c] -> [p, t, c]
        def dtile(w):
            return np.ascontiguousarray(
                w.reshape(DT, 128, w.shape[1]).transpose(1, 0, 2))
        wqk = dtile(np.concatenate(
            [wq[:, 0:128], wk[:, 0:128], wq[:, 128:192], wk[:, 128:192]],
            axis=1))
        bqk = np.concatenate([bq[hsl][0:128], bk[hsl][0:128],
                              bq[hsl][128:192], bk[hsl][128:192]])
        m = {
            "x": np.ascontiguousarray(X[b]).astype(ml_dtypes.bfloat16),
            "wqk": wqk.astype(ml_dtypes.bfloat16),
            "bqk": np.ascontiguousarray(bqk).astype(np.float32),
            "wv": dtile(wv).astype(ml_dtypes.bfloat16),
            "bv": np.ascontiguousarray(bv[hsl]).astype(ml_dtypes.bfloat16),
            "wo": np.ascontiguousarray(
                Wo[hsl, :].reshape(HPC, 64, D).transpose(1, 0, 2)).astype(
                    ml_dtypes.bfloat16),
        }
        if with_mask:
            m["mask"] = np.ascontiguousarray(mask[b]).astype(np.float32)
        in_maps.append(m)
    return in_maps


def kernel(X, mask, Wq, bq, Wk, bk, Wv, bv, Wo, bo):
    from concourse.bass_utils import run_bass_kernel_spmd

    X = np.asarray(X, dtype=np.float32)
    mask = np.asarray(mask, dtype=np.float32)
    Wq, bq = np.asarray(Wq, np.float32), np.asarray(bq, np.float32)
    Wk, bk = np.asarray(Wk, np.float32), np.asarray(bk, np.float32)
    Wv, bv = np.asarray(Wv, np.float32), np.asarray(bv, np.float32)
    Wo, bo = np.asarray(Wo, np.float32), np.asarray(bo, np.float32)

    with_mask = not np.all(mask == 1.0)
    nc = _get_nc(S, with_mask)
    in_maps = make_in_maps(X, mask, Wq, bq, Wk, bk, Wv, bv, Wo, bo, with_mask)
    res = run_bass_kernel_spmd(nc, in_maps, list(range(NCORES))).results
    out = np.zeros((B, S, D), dtype=np.float32)
    for c in range(NCORES):
        out[c // 4] += res[c]["out"].astype(np.float32)
    out += bo
    return out
